# revision 7
# baseline (speedup 1.0000x reference)
"""Causal single-head attention (B=4, S=2048, D=1024, fp32) on 8 TRN2 cores.

Sharding: core = (batch b, half h), so each batch owns a core PAIR.
Queries: each core takes 4 contiguous 256-query blocks chosen so both halves
have identical causal block structure (key-block counts [1,2,3,4] in local
block order) -> identical SPMD instruction stream:

    h=0 -> query blocks at rows {0, 768, 1024, 1792}
    h=1 -> query blocks at rows {256, 512, 1280, 1536}

K/V projections are deduplicated across the pair: each core projects only its
own 1024-key half of kT and v, then the pair exchanges halves with an
AllGather (replica groups [[0,1],[2,3],[4,5],[6,7]]) through DRAM bounce
tiles.  v lives in DRAM (streamed back per key tile during attention).

Per-core device kernel (all matmuls fp32r = FP22 mantissa, fp32 PSUM accum):
  For each query block j (256 q), key blocks 0..j (512 keys each):
    scoresT[k, q] = kT^T q           (keys on partitions)
    += additive causal mask          (host input, diagonal block only)
    attnT = exp(scoresT / 32)        (no max-subtraction: |s/32| <~ 6)
    AV[q, d] += attnT^T v, rowsum[q] += attnT^T ones   (PE matmuls)
  out[q, :] = AV[q, :] / rowsum[q]
"""

import os
import subprocess
import sys
import tempfile

import numpy as np

B, S, D = 4, 2048, 1024
P = 128
NSL = D // P          # 8 contraction subtiles of 128
NQB = 4               # local query blocks (256 queries each)
QB_MAP = {0: (0, 768, 1024, 1792), 1: (256, 512, 1280, 1536)}
MASK_NEG = -1.0e9
SCALE = 1.0 / np.sqrt(np.float32(D))

_CACHE = {}


def _build_nc(reps=1):
    from contextlib import ExitStack

    import concourse.mybir as mybir
    from concourse import bacc
    from concourse.tile import TileContext

    f32 = mybir.dt.float32
    f32r = mybir.dt.float32r
    Exp = mybir.ActivationFunctionType.Exp

    nc = bacc.Bacc("TRN2", target_bir_lowering=False, debug=False,
                   enable_asserts=False, num_devices=8)
    xTh_d = nc.dram_tensor("xTh", [D, 1024], f32r, kind="ExternalInput").ap()
    xTq_d = nc.dram_tensor("xTq", [D, 1024], f32r, kind="ExternalInput").ap()
    wq_d = nc.dram_tensor("wq", [D, D], f32r, kind="ExternalInput").ap()
    wk_d = nc.dram_tensor("wk", [D, D], f32r, kind="ExternalInput").ap()
    wv_d = nc.dram_tensor("wv", [D, D], f32r, kind="ExternalInput").ap()
    mask_d = nc.dram_tensor("mask", [NQB, 2, P, 2, 256], f32,
                            kind="ExternalInput").ap()
    ones_d = nc.dram_tensor("onesd", [P, 2], f32r, kind="ExternalInput").ap()
    out_d = nc.dram_tensor("out", [1024, D], f32, kind="ExternalOutput").ap()

    groups = [[0, 1], [2, 3], [4, 5], [6, 7]]

    with TileContext(nc) as tc, ExitStack() as ctx:
        persist = ctx.enter_context(tc.tile_pool(name="persist", bufs=1))
        qT = persist.tile([P, NSL, 1024], f32r, tag="qT", name="qT")
        kT = persist.tile([P, NSL, 2048], f32r, tag="kT", name="kT")
        ones = persist.tile([P, 2], f32r, tag="ones", name="ones")
        nc.sync.dma_start(out=ones, in_=ones_d)
        dram = ctx.enter_context(tc.tile_pool(name="dram", bufs=1, space="DRAM"))
        kb_in = dram.tile([P, NSL, 1024], f32r, tag="kbi", name="kb_in")
        kb_out = dram.tile([2, P, NSL, 1024], f32r, tag="kbo", name="kb_out")
        vb_in = dram.tile([8, P, 1024], f32r, tag="vbi", name="vb_in")
        vb_out = dram.tile([16, P, 1024], f32r, tag="vbo", name="vb_out")

        for _rep in range(reps):
            # ---- K-half projection: kb_in[d_out, k_local] = Wk^T @ xTh ----
            with tc.tile_pool(name="pk", bufs=1) as pk, \
                 tc.tile_pool(name="pkx", bufs=2) as pkx, \
                 tc.tile_pool(name="pko", bufs=3) as pko, \
                 tc.tile_pool(name="psk", bufs=4, space="PSUM") as psk:
                w = pk.tile([P, NSL, 1024], f32r, tag="w", name="wk_t")
                nc.sync.dma_start(out=w,
                                  in_=wk_d.rearrange("(s p) o -> p s o", p=P))
                xTh_r = xTh_d.rearrange("(s p) k -> p s k", p=P)
                for g in range(2):
                    xg = pkx.tile([P, NSL, 512], f32r, tag="xg", name="xg_t")
                    nc.sync.dma_start(out=xg,
                                      in_=xTh_r[:, :, g * 512:(g + 1) * 512])
                    for c in range(NSL):
                        ps = psk.tile([P, 512], f32, tag="ps", name="ps_k")
                        for s in range(NSL):
                            nc.tensor.matmul(ps, w[:, s, c * P:(c + 1) * P],
                                             xg[:, s, :],
                                             start=(s == 0), stop=(s == NSL - 1))
                        ko = pko.tile([P, 512], f32r, tag="ko", name="ko_t")
                        nc.vector.tensor_copy(out=ko, in_=ps)
                        nc.sync.dma_start(
                            out=kb_in[:, c, g * 512:(g + 1) * 512], in_=ko)
            nc.gpsimd.collective_compute(
                "AllGather", mybir.AluOpType.bypass, replica_groups=groups,
                ins=[kb_in], outs=[kb_out])
            for hh in range(2):
                nc.sync.dma_start(out=kT[:, :, hh * 1024:(hh + 1) * 1024],
                                  in_=kb_out[hh])

            # ---- V-half projection: vb_in[k_local, d_out] = x Wv ----
            with tc.tile_pool(name="pv", bufs=1) as pv, \
                 tc.tile_pool(name="pvx", bufs=3) as pvx, \
                 tc.tile_pool(name="pvo", bufs=3) as pvo, \
                 tc.tile_pool(name="psv", bufs=4, space="PSUM") as psv:
                w = pv.tile([P, NSL, 1024], f32r, tag="w", name="wv_t")
                nc.sync.dma_start(out=w,
                                  in_=wv_d.rearrange("(s p) o -> p s o", p=P))
                xTh_r = xTh_d.rearrange("(s p) k -> p s k", p=P)
                for kt in range(8):
                    xk = pvx.tile([P, NSL, P], f32r, tag="xk", name="xk_t")
                    nc.sync.dma_start(out=xk,
                                      in_=xTh_r[:, :, kt * P:(kt + 1) * P])
                    vo = pvo.tile([P, 1024], f32r, tag="vo", name="vo_t")
                    for hh in range(2):
                        ps = psv.tile([P, 512], f32, tag="ps", name="ps_v")
                        for s in range(NSL):
                            nc.tensor.matmul(ps, xk[:, s, :],
                                             w[:, s, hh * 512:(hh + 1) * 512],
                                             start=(s == 0), stop=(s == NSL - 1))
                        nc.scalar.copy(out=vo[:, hh * 512:(hh + 1) * 512],
                                       in_=ps)
                    nc.sync.dma_start(out=vb_in[kt], in_=vo)
            nc.gpsimd.collective_compute(
                "AllGather", mybir.AluOpType.bypass, replica_groups=groups,
                ins=[vb_in], outs=[vb_out])

            # ---- Q projection: qT[d_out, q] = Wq^T @ xTq ----
            with tc.tile_pool(name="pq", bufs=1) as pq, \
                 tc.tile_pool(name="psq", bufs=4, space="PSUM") as psq:
                w = pq.tile([P, NSL, 1024], f32r, tag="w", name="wq_t")
                nc.sync.dma_start(out=w,
                                  in_=wq_d.rearrange("(s p) o -> p s o", p=P))
                xq = pq.tile([P, NSL, 1024], f32r, tag="xq", name="xq_t")
                nc.sync.dma_start(out=xq,
                                  in_=xTq_d.rearrange("(s p) q -> p s q", p=P))
                for c in range(NSL):
                    for g in range(2):
                        ps = psq.tile([P, 512], f32, tag="ps", name="ps_q")
                        for s in range(NSL):
                            nc.tensor.matmul(ps, w[:, s, c * P:(c + 1) * P],
                                             xq[:, s, g * 512:(g + 1) * 512],
                                             start=(s == 0), stop=(s == NSL - 1))
                        nc.vector.tensor_copy(
                            out=qT[:, c, g * 512:(g + 1) * 512], in_=ps)

            # ---- Attention ----
            with tc.tile_pool(name="avv", bufs=3) as vvp, \
                 tc.tile_pool(name="aat", bufs=3) as atp, \
                 tc.tile_pool(name="amk", bufs=2) as mkp, \
                 tc.tile_pool(name="aot", bufs=2) as otp, \
                 tc.tile_pool(name="asm", bufs=4) as smp, \
                 tc.tile_pool(name="pssc", bufs=2, space="PSUM") as pssc, \
                 tc.tile_pool(name="psav", bufs=4, space="PSUM") as psav, \
                 tc.tile_pool(name="pssm", bufs=2, space="PSUM") as pssm:
                for j in range(NQB):
                    npair = 2 * (j + 1)      # 256-key pairs: key blocks 0..j
                    av = [psav.tile([P, 512], f32, tag="av", name=f"av_{j}_{i}")
                          for i in range(4)]             # [qsub*2 + dhalf]
                    sums = [pssm.tile([P, 2], f32, tag="sums",
                                      name=f"sums_{j}_{qs}") for qs in range(2)]
                    for pr in range(npair):
                        sc = pssc.tile([P, 2, 256], f32, tag="sc",
                                       name=f"sc_{j}_{pr}")
                        for t in range(2):
                            ktile = 2 * pr + t
                            for s in range(NSL):
                                nc.tensor.matmul(
                                    sc[:, t, :],
                                    kT[:, s, ktile * P:(ktile + 1) * P],
                                    qT[:, s, j * 256:(j + 1) * 256],
                                    start=(s == 0), stop=(s == NSL - 1))
                        if pr >= npair - 2:  # diagonal 512-key block: mask
                            mk = mkp.tile([P, 2, 256], f32, tag="mk",
                                          name=f"mk_{j}_{pr}")
                            nc.sync.dma_start(out=mk,
                                              in_=mask_d[j, pr - (npair - 2)])
                            nc.vector.tensor_add(out=sc, in0=sc, in1=mk)
                        at = atp.tile([P, 2, 256], f32r, tag="at",
                                      name=f"at_{j}_{pr}")
                        nc.scalar.activation(out=at, in_=sc, func=Exp,
                                             scale=float(SCALE))
                        for t in range(2):
                            ktile = 2 * pr + t
                            vv = vvp.tile([P, 1024], f32r, tag="vv",
                                          name=f"vv_{j}_{pr}_{t}")
                            nc.sync.dma_start(out=vv, in_=vb_out[ktile])
                            st = (pr == 0 and t == 0)
                            sp = (pr == npair - 1 and t == 1)
                            for qs in range(2):
                                lhs = at[:, t, qs * P:(qs + 1) * P]
                                for hh in range(2):
                                    nc.tensor.matmul(
                                        av[qs * 2 + hh], lhs,
                                        vv[:, hh * 512:(hh + 1) * 512],
                                        start=st, stop=sp)
                                nc.tensor.matmul(sums[qs], lhs, ones,
                                                 start=st, stop=sp)
                    for qs in range(2):
                        rec = smp.tile([P, 1], f32, tag="rec",
                                       name=f"rec_{j}_{qs}")
                        nc.vector.reciprocal(out=rec, in_=sums[qs][:, 0:1])
                        ot = otp.tile([P, 1024], f32, tag="ot",
                                      name=f"ot_{j}_{qs}")
                        for hh in range(2):
                            nc.vector.tensor_scalar_mul(
                                ot[:, hh * 512:(hh + 1) * 512],
                                av[qs * 2 + hh], rec)
                        row = j * 256 + qs * P
                        nc.sync.dma_start(out=out_d[row:row + P, :], in_=ot)

    nc.compile()
    return nc


def _make_mask(h):
    m = np.zeros((NQB, 2, P, 2, 256), np.float32)
    for j, qb in enumerate(QB_MAP[h]):
        for prr in range(2):
            for t in range(2):
                keys = 512 * j + 256 * prr + 128 * t + np.arange(P)
                qs_ = qb + np.arange(256)
                m[j, prr, :, t, :] = np.where(keys[:, None] <= qs_[None, :],
                                              0.0, MASK_NEG)
    return m


def build_in_maps(x, Wq, Wk, Wv):
    x = np.ascontiguousarray(np.asarray(x, dtype=np.float32))
    Wq = np.ascontiguousarray(np.asarray(Wq, dtype=np.float32))
    Wk = np.ascontiguousarray(np.asarray(Wk, dtype=np.float32))
    Wv = np.ascontiguousarray(np.asarray(Wv, dtype=np.float32))
    masks = {h: _make_mask(h) for h in (0, 1)}
    onesd = np.ones((P, 2), np.float32)
    in_maps = []
    for b in range(B):
        xt = np.ascontiguousarray(x[b].T)
        for h in range(2):
            xth = np.ascontiguousarray(xt[:, h * 1024:(h + 1) * 1024])
            xtq = np.ascontiguousarray(np.concatenate(
                [xt[:, qb:qb + 256] for qb in QB_MAP[h]], axis=1))
            in_maps.append({"xTh": xth, "xTq": xtq, "wq": Wq, "wk": Wk,
                            "wv": Wv, "mask": masks[h], "onesd": onesd})
    return in_maps


def assemble_out(results):
    out = np.empty((B, S, D), np.float32)
    for b in range(B):
        for h in range(2):
            o = results[2 * b + h]["out"]
            for ji, qb in enumerate(QB_MAP[h]):
                out[b, qb:qb + 256] = o[ji * 256:(ji + 1) * 256]
    return out


def get_nc():
    nc = _CACHE.get("nc")
    if nc is None:
        nc = _build_nc()
        _CACHE["nc"] = nc
    return nc


def _run_once(x, Wq, Wk, Wv):
    from concourse.bass_utils import run_bass_kernel_spmd

    nc = get_nc()
    in_maps = build_in_maps(x, Wq, Wk, Wv)
    res = run_bass_kernel_spmd(nc, in_maps, core_ids=list(range(8)))
    return assemble_out(res.results)


def _run_subprocess(x, Wq, Wk, Wv):
    """Fallback: run in a fresh process (a wedged device session heals on
    process close)."""
    td = tempfile.mkdtemp()
    inp = os.path.join(td, "in.npz")
    outp = os.path.join(td, "out.npy")
    np.savez(inp, x=x, Wq=Wq, Wk=Wk, Wv=Wv)
    code = (
        "import numpy as np\n"
        "import sys\n"
        "sys.path.insert(0, %r)\n"
        "import kernel\n"
        "d = np.load(%r)\n"
        "out = kernel._run_once(d['x'], d['Wq'], d['Wk'], d['Wv'])\n"
        "np.save(%r, out)\n"
    ) % (os.path.dirname(os.path.abspath(__file__)), inp, outp)
    subprocess.run([sys.executable, "-c", code], check=True, timeout=1800)
    return np.load(outp)


def kernel(x, Wq, Wk, Wv):
    try:
        return _run_once(x, Wq, Wk, Wv)
    except Exception as e:
        print(f"kernel: in-process run failed ({type(e).__name__}: {e}); "
              f"retrying in a fresh process", file=sys.stderr)
        try:
            return _run_subprocess(x, Wq, Wk, Wv)
        except Exception:
            import time
            time.sleep(30)
            return _run_subprocess(x, Wq, Wk, Wv)
